# revision 2
# baseline (speedup 1.0000x reference)
"""ChebConv (R=4) Trainium2 kernel: 8-core batch-parallel TRUE-SPARSE SpMM.

Sharding: batch dim B=16 -> 2 batches/core (F=256 features/core), zero
collectives. Per core, per Chebyshev step k=1..3:
  - dma_gather (SWDGE) pulls x[col[e]] rows (fp8e3, 256B) from HBM in
    edge-major layout [128, nch, 256] (edge e -> partition e%128, chunk e//128)
  - DVE builds one-hot*val scatter blocks S [128e, 128d] (bf16) from
    dest-local ids + vals via iota/is_equal/mult
  - PE accumulates y[dest,:] = sum_chunks S^T @ msg into PSUM per dest block
  - combine: x_k = 2*y - x_{k-2} (bf16), cast to fp8e3 for next gather,
    PE-transpose to f-major for the final einsum
Final einsum: out[b,o,v] = sum_r W_r^T x_r^T + bias on PE.
Edges are host-sorted by dest block (128 rows) and padded per block to a
multiple of 128 with val=0 edges.
"""
import sys

sys.path.insert(0, '/opt/trn_rl_repo')
sys.path.insert(0, '/opt/pypackages')

import numpy as np
import ml_dtypes

import concourse.bacc as bacc
import concourse.mybir as mybir
import concourse.tile as tile
from concourse import bass_utils

BF16 = mybir.dt.bfloat16
FP8 = mybir.dt.float8e3
F32 = mybir.dt.float32
I16 = mybir.dt.int16

NP_FP8 = ml_dtypes.float8_e3m4

N_CORES = 8
N_QUEUES = 1
SLICE_CH = 64          # chunks (of 128 edges) per gather slice
STORE_GRP = 4          # dest blocks per batched store DMA


def make_plan(lap_rows, lap_cols, lap_vals, V):
    """Host-side: dest-block-sort edges, pad per block to x128.

    Returns dict with padded edge arrays in kernel layouts plus the
    chunk->block structure needed at build time.
    """
    lap_rows = np.asarray(lap_rows)
    lap_cols = np.asarray(lap_cols)
    lap_vals = np.asarray(lap_vals, np.float32)
    NB = (V + 127) // 128
    VP = NB * 128
    blk = lap_rows // 128
    order = np.argsort(blk, kind="stable")
    rows_s = lap_rows[order]
    cols_s = lap_cols[order]
    vals_s = lap_vals[order]
    counts = np.bincount(blk, minlength=NB)
    nch = np.maximum((counts + 127) // 128, 1)    # chunks per block, >=1
    EP = int(nch.sum()) * 128

    gcol = np.zeros(EP, np.int16)
    dloc = np.zeros(EP, np.int16)
    sval = np.zeros(EP, np.float32)
    src_off = 0
    dst_off = 0
    cstart = np.zeros(NB, np.int64)
    coff = 0
    for I in range(NB):
        n = int(counts[I])
        gcol[dst_off:dst_off + n] = cols_s[src_off:src_off + n]
        dloc[dst_off:dst_off + n] = rows_s[src_off:src_off + n] - I * 128
        sval[dst_off:dst_off + n] = vals_s[src_off:src_off + n]
        cstart[I] = coff
        coff += int(nch[I])
        src_off += n
        dst_off += int(nch[I]) * 128
    NCH_TOT = coff
    assert dst_off == EP == NCH_TOT * 128

    # block id of each chunk
    chunk_blk = np.repeat(np.arange(NB), nch)

    # gather index tensor: position i -> (partition i%16 (replicated to 128),
    # free i//16)
    gidx = gcol.reshape(-1, 16).T.astype(np.int16)          # [16, EP/16]
    gidx = np.tile(gidx, (8, 1))                            # [128, EP/16]
    # dloc/sval: position i -> (partition i%128, chunk i//128)
    dloc_t = dloc.reshape(-1, 128).T.astype(ml_dtypes.bfloat16)  # [128, NCH]
    sval_t = sval.reshape(-1, 128).T.astype(ml_dtypes.bfloat16)  # [128, NCH]

    return dict(NB=NB, VP=VP, EP=EP, NCH=NCH_TOT, nch=nch, cstart=cstart,
                chunk_blk=chunk_blk, gidx=gidx, dloc=dloc_t, sval=sval_t)


def build_kernel(V, plan, R=4, BL=2, CIN=128, COUT=128):
    F = BL * CIN                   # 256
    NB, VP, EP, NCH = plan["NB"], plan["VP"], plan["EP"], plan["NCH"]
    nch, cstart, chunk_blk = plan["nch"], plan["cstart"], plan["chunk_blk"]
    NE_CHUNK = (V + 511) // 512    # einsum v-chunks (512 wide, last partial)

    nc = bacc.Bacc("TRN2", target_bir_lowering=False, debug=False,
                   num_swdge_queues=N_QUEUES)

    # ---- DRAM tensors ----
    xg0 = nc.dram_tensor("xg0", [VP, F], FP8, kind="ExternalInput")
    x0nm = nc.dram_tensor("x0nm", [VP, F], BF16, kind="ExternalInput")
    x0T = nc.dram_tensor("x0T", [F, VP], BF16, kind="ExternalInput")
    gidx = nc.dram_tensor("gidx", [128, EP // 16], I16, kind="ExternalInput")
    dlocd = nc.dram_tensor("dlocd", [128, NCH], BF16, kind="ExternalInput")
    svald = nc.dram_tensor("svald", [128, NCH], BF16, kind="ExternalInput")
    iotad = nc.dram_tensor("iotad", [128, SLICE_CH * 128], BF16,
                           kind="ExternalInput")
    identd = nc.dram_tensor("identd", [128, 128], BF16, kind="ExternalInput")
    wt = nc.dram_tensor("wt", [CIN, R, COUT], BF16, kind="ExternalInput")
    biasv = nc.dram_tensor("biasv", [COUT, 1], F32, kind="ExternalInput")
    yout = nc.dram_tensor("yout", [BL, COUT, V], F32, kind="ExternalOutput")

    xg = [nc.dram_tensor(f"xg{k}", [VP, F], FP8) for k in (1, 2)]
    xnm = [nc.dram_tensor(f"x{k}nm", [VP, F], BF16) for k in (1, 2)]
    xT = [nc.dram_tensor(f"x{k}T", [F, VP], BF16) for k in (1, 2, 3)]

    with tile.TileContext(nc, trace_sim=False) as tc:
        with (
            tc.tile_pool(name="res", bufs=1) as res,       # resident tables
            tc.tile_pool(name="ip", bufs=3) as ip,         # gather idx slices
            tc.tile_pool(name="gp", bufs=2) as gp,         # gathered msgs
            tc.tile_pool(name="sp", bufs=2) as sp,         # S blocks
            tc.tile_pool(name="pp", bufs=3, space="PSUM") as pp,   # y accum
            tc.tile_pool(name="tp", bufs=2, space="PSUM") as tp,   # transposes
            tc.tile_pool(name="cb", bufs=3) as cb,         # combine tiles
            tc.tile_pool(name="ep", bufs=8) as ep,         # einsum x tiles
            tc.tile_pool(name="pep", bufs=2, space="PSUM") as pep,  # einsum psum
            tc.tile_pool(name="eo", bufs=2) as eo,         # einsum out tiles
        ):
            # ---- resident tables ----
            dloc_sb = res.tile([128, NCH], BF16)
            nc.sync.dma_start(dloc_sb[:], dlocd[:])
            sval_sb = res.tile([128, NCH], BF16)
            nc.sync.dma_start(sval_sb[:], svald[:])
            iota_sb = res.tile([128, SLICE_CH * 128], BF16)
            nc.sync.dma_start(iota_sb[:], iotad[:])
            ident_sb = res.tile([128, 128], BF16)
            nc.sync.dma_start(ident_sb[:], identd[:])
            wts = res.tile([128, R, COUT], BF16)
            nc.sync.dma_start(wts[:], wt[:])
            bias_sb = res.tile([128, 1], F32)
            nc.sync.dma_start(bias_sb[:], biasv[:])

            gsrc = [xg0, xg[0], xg[1]]
            prevnm = [None, x0nm, xnm[0]]
            xnm_dst = [xnm[0], xnm[1], None]
            xg_dst = [xg[0], xg[1], None]
            xT_dst = [xT[0], xT[1], xT[2]]

            n_slice = (NCH + SLICE_CH - 1) // SLICE_CH

            for k in (0, 1, 2):       # chebyshev steps 1..3
                src_rows = gsrc[k][:]          # [VP, F] fp8
                # per-STORE_GRP group tiles (batched stores)
                grp_nm = None
                grp_f8 = None
                grp_tr = None
                for s in range(n_slice):
                    c0 = s * SLICE_CH
                    ncs = min(SLICE_CH, NCH - c0)
                    ne = ncs * 128
                    # gather idx slice
                    it = ip.tile([128, SLICE_CH * 8], I16, tag="idx")
                    nc.sync.dma_start(it[:, :ncs * 8],
                                      gidx[:, c0 * 8:c0 * 8 + ncs * 8])
                    # gather messages
                    mt = gp.tile([128, SLICE_CH, F], FP8, tag="msg")
                    nc.gpsimd.dma_gather(
                        mt[:, :ncs, :], src_rows, it[:, :ncs * 8],
                        num_idxs=ne, num_idxs_reg=ne, elem_size=F,
                        single_packet=False, queue_num=s % N_QUEUES)
                    # build S blocks for this slice
                    st = sp.tile([128, SLICE_CH, 128], BF16, tag="sblk")
                    nc.vector.tensor_tensor(
                        out=st[:, :ncs, :],
                        in0=dloc_sb[:, c0:c0 + ncs, None].to_broadcast(
                            [128, ncs, 128]),
                        in1=iota_sb[:].rearrange("p (c d) -> p c d", d=128)[
                            :, :ncs, :],
                        op=mybir.AluOpType.is_equal)
                    nc.vector.tensor_tensor(
                        out=st[:, :ncs, :],
                        in0=st[:, :ncs, :],
                        in1=sval_sb[:, c0:c0 + ncs, None].to_broadcast(
                            [128, ncs, 128]),
                        op=mybir.AluOpType.mult)
                    # scatter matmuls
                    for cl in range(ncs):
                        c = c0 + cl
                        I = int(chunk_blk[c])
                        first = (c == cstart[I])
                        last = (c == cstart[I] + nch[I] - 1)
                        if first:
                            ps = pp.tile([128, F], F32, tag="yac",
                                         name=f"yac_{k}_{I}")
                        nc.tensor.matmul(
                            ps[:], lhsT=st[:, cl, :], rhs=mt[:, cl, :],
                            start=first, stop=last)
                        if not last:
                            continue
                        # ---- combine block I ----
                        g = I % STORE_GRP
                        if g == 0:
                            ng = min(STORE_GRP, NB - I)
                            grp_nm = cb.tile([128, STORE_GRP, F], BF16,
                                             tag="gnm")
                            if k < 2:
                                grp_f8 = cb.tile([128, STORE_GRP, F], FP8,
                                                 tag="gf8")
                            grp_tr = cb.tile([128, STORE_GRP, 2, 128], BF16,
                                             tag="gtr")
                            if k > 0:
                                grp_pv = cb.tile([128, STORE_GRP, F], BF16,
                                                 tag="prev")
                                nc.sync.dma_start(
                                    grp_pv[:, :ng, :],
                                    prevnm[k][I * 128:I * 128 + ng * 128,
                                              :].rearrange(
                                        "(g p) f -> p g f", p=128))
                        if k == 0:
                            nc.scalar.activation(
                                grp_nm[:, g, :], ps[:],
                                mybir.ActivationFunctionType.Copy)
                        else:
                            t2 = cb.tile([128, F], BF16, tag="twoy")
                            nc.vector.tensor_scalar_mul(t2[:], ps[:], 2.0)
                            nc.vector.tensor_tensor(
                                out=grp_nm[:, g, :], in0=t2[:],
                                in1=grp_pv[:, g, :],
                                op=mybir.AluOpType.subtract)
                        if k < 2:
                            nc.scalar.activation(
                                grp_f8[:, g, :], grp_nm[:, g, :],
                                mybir.ActivationFunctionType.Copy)
                        for h in (0, 1):
                            tps = tp.tile([128, 128], BF16, tag="tps")
                            nc.tensor.transpose(
                                tps[:], grp_nm[:, g, h * 128:(h + 1) * 128],
                                ident_sb[:])
                            nc.scalar.activation(
                                grp_tr[:, g, h, :], tps[:],
                                mybir.ActivationFunctionType.Copy)
                        if g == ng - 1:
                            # batched stores for blocks I-g .. I
                            I0 = I - g
                            rsl = slice(I0 * 128, I0 * 128 + ng * 128)
                            if k < 2:
                                nc.scalar.dma_start(
                                    xnm_dst[k][rsl, :].rearrange(
                                        "(g p) f -> p g f", p=128),
                                    grp_nm[:, :ng, :])
                                nc.scalar.dma_start(
                                    xg_dst[k][rsl, :].rearrange(
                                        "(g p) f -> p g f", p=128),
                                    grp_f8[:, :ng, :])
                            for h in (0, 1):
                                nc.scalar.dma_start(
                                    xT_dst[k][h * 128:(h + 1) * 128,
                                              rsl].rearrange(
                                        "f (g d) -> f g d", d=128),
                                    grp_tr[:, :ng, h, :])

            # ---- einsum: out[b,o,v] = sum_r W_r^T @ x_r^T + bias ----
            xTs = [x0T, xT[0], xT[1], xT[2]]
            for b in range(BL):
                for chn in range(NE_CHUNK):
                    v0 = chn * 512
                    w = min(512, VP - v0)
                    wout = min(512, V - v0)
                    eps = pep.tile([128, 512], F32, tag="eps")
                    for r in range(R):
                        xt = ep.tile([128, 512], BF16, tag="ext")
                        nc.sync.dma_start(
                            xt[:, :w],
                            xTs[r][b * 128:(b + 1) * 128, v0:v0 + w])
                        nc.tensor.matmul(
                            eps[:, :w], lhsT=wts[:, r, :], rhs=xt[:, :w],
                            start=(r == 0), stop=(r == R - 1))
                    ob = eo.tile([128, 512], F32, tag="eob")
                    nc.vector.tensor_scalar_add(ob[:, :w], eps[:, :w],
                                                bias_sb[:])
                    nc.scalar.dma_start(yout[b, :, v0:v0 + wout],
                                        ob[:, :wout])

    nc.compile()
    return nc


def prep_inputs(x, weight, bias, lap_vals, lap_rows, lap_cols, plan):
    B, CIN, V = x.shape
    R = weight.shape[0]
    BL = B // N_CORES
    F = BL * CIN
    VP = plan["VP"]

    wt = np.ascontiguousarray(
        np.asarray(weight, np.float32).transpose(1, 0, 2)
    ).astype(ml_dtypes.bfloat16)                      # [CIN, R, COUT]
    biasv = np.asarray(bias, np.float32).reshape(-1, 1)
    iota = np.tile(np.arange(128, dtype=np.float32), SLICE_CH)
    iota = np.broadcast_to(iota, (128, SLICE_CH * 128)).astype(
        ml_dtypes.bfloat16)
    ident = np.eye(128, dtype=np.float32).astype(ml_dtypes.bfloat16)

    xf = np.asarray(x, np.float32)
    in_maps = []
    for c in range(N_CORES):
        xs = xf[c * BL:(c + 1) * BL]                    # (BL, CIN, V)
        x0 = np.transpose(xs, (2, 0, 1)).reshape(V, F)  # (V, F)
        x0p = np.zeros((VP, F), np.float32)
        x0p[:V] = x0
        x0b = x0p.astype(ml_dtypes.bfloat16)
        in_maps.append({
            "xg0": x0b.astype(NP_FP8),
            "x0nm": x0b,
            "x0T": np.ascontiguousarray(x0b.T),
            "gidx": plan["gidx"],
            "dlocd": plan["dloc"],
            "svald": plan["sval"],
            "iotad": np.ascontiguousarray(iota),
            "identd": ident,
            "wt": wt,
            "biasv": biasv,
        })
    return in_maps


_CACHE = {}


def get_built(x, weight, bias, lap_vals, lap_rows, lap_cols):
    V = x.shape[2]
    key = (V, len(lap_vals))
    if key not in _CACHE:
        plan = make_plan(lap_rows, lap_cols, lap_vals, V)
        nc = build_kernel(V, plan)
        _CACHE[key] = (nc, plan)
    return _CACHE[key]


def kernel(x, weight, bias, lap_vals, lap_rows, lap_cols):
    B, CIN, V = x.shape
    nc, plan = get_built(x, weight, bias, lap_vals, lap_rows, lap_cols)
    in_maps = prep_inputs(x, weight, bias, lap_vals, lap_rows, lap_cols, plan)
    res = bass_utils.run_bass_kernel_spmd(
        nc, in_maps, core_ids=list(range(N_CORES)))
    out = np.concatenate([res.results[c]["yout"] for c in range(N_CORES)],
                         axis=0)
    return out.astype(np.float32)


if __name__ == "__main__":
    V, NNZ, B, CIN, COUT, R = 1024, 32768, 16, 128, 128, 4
    rng = np.random.default_rng(0)
    x = rng.standard_normal((B, CIN, V)).astype(np.float32)
    weight = (rng.standard_normal((R, CIN, COUT)) *
              np.sqrt(2.0 / (R * CIN))).astype(np.float32)
    bias = np.full((COUT,), 0.01, np.float32)
    lap_vals = (rng.standard_normal(NNZ) / 32.0).astype(np.float32)
    lap_rows = rng.integers(0, V, NNZ).astype(np.int32)
    lap_cols = rng.integers(0, V, NNZ).astype(np.int32)

    def ref(x, weight, bias, lv, lr, lc):
        Vd_ = x.shape[2]
        L = np.zeros((Vd_, Vd_), np.float64)
        np.add.at(L, (lr, lc), lv.astype(np.float64))
        x0 = np.transpose(x, (2, 0, 1)).reshape(Vd_, -1).astype(np.float64)
        xs = [x0, L @ x0]
        for _ in range(R - 2):
            xs.append(2.0 * (L @ xs[-1]) - xs[-2])
        xs = np.stack(xs).reshape(R, Vd_, B, CIN)
        out = np.einsum('rvbi,rio->vbo', xs, weight.astype(np.float64))
        out = out + bias
        return np.transpose(out, (1, 2, 0)).astype(np.float32)

    expected = ref(x, weight, bias, lap_vals, lap_rows, lap_cols)
    got = kernel(x, weight, bias, lap_vals, lap_rows, lap_cols)
    err = np.abs(got - expected)
    scale = np.abs(expected).max()
    print("max abs err:", err.max(), "scale:", scale,
          "rel:", err.max() / scale)


# revision 3
# speedup vs baseline: 2.7272x; 2.7272x over previous
"""ChebConv (R=4) Trainium2 kernel: 8-core batch-parallel TRUE-SPARSE SpMM.

Sharding: batch dim B=16 -> 2 batches/core (F=256 features/core), zero
collectives. Per core, per Chebyshev step k=1..3:
  - dma_gather (SWDGE) pulls x[col[e]] rows (fp8e3, 256B) from HBM in
    edge-major layout [128, nch, 256] (edge e -> partition e%128, chunk e//128)
  - DVE builds one-hot*val scatter blocks S [128e, 128d] (bf16) from
    dest-local ids + vals via iota/is_equal/mult
  - PE accumulates y[dest,:] = sum_chunks S^T @ msg into PSUM per dest block
  - combine: x_k = 2*y - x_{k-2} (bf16), cast to fp8e3 for next gather,
    PE-transpose to f-major for the final einsum
Final einsum: out[b,o,v] = sum_r W_r^T x_r^T + bias on PE.
Edges are host-sorted by dest block (128 rows) and padded per block to a
multiple of 128 with val=0 edges.
"""
import sys

sys.path.insert(0, '/opt/trn_rl_repo')
sys.path.insert(0, '/opt/pypackages')

import numpy as np
import ml_dtypes

import concourse.bacc as bacc
import concourse.mybir as mybir
import concourse.tile as tile
from concourse import bass_utils

BF16 = mybir.dt.bfloat16
FP8 = mybir.dt.float8e3
F32 = mybir.dt.float32
I16 = mybir.dt.int16

NP_FP8 = ml_dtypes.float8_e3m4

N_CORES = 8
N_QUEUES = 4
SLICE_CH = 32          # chunks (of 128 edges) per gather slice
STORE_GRP = 4          # dest blocks per batched store DMA


def make_plan(lap_rows, lap_cols, lap_vals, V):
    """Host-side: dest-block-sort edges, pad per block to x128.

    Returns dict with padded edge arrays in kernel layouts plus the
    chunk->block structure needed at build time.
    """
    lap_rows = np.asarray(lap_rows)
    lap_cols = np.asarray(lap_cols)
    lap_vals = np.asarray(lap_vals, np.float32)
    NB = (V + 127) // 128
    VP = NB * 128
    blk = lap_rows // 128
    order = np.argsort(blk, kind="stable")
    rows_s = lap_rows[order]
    cols_s = lap_cols[order]
    vals_s = lap_vals[order]
    counts = np.bincount(blk, minlength=NB)
    nch = np.maximum((counts + 127) // 128, 1)    # chunks per block, >=1
    EP = int(nch.sum()) * 128

    gcol = np.zeros(EP, np.int16)
    dloc = np.zeros(EP, np.int16)
    sval = np.zeros(EP, np.float32)
    src_off = 0
    dst_off = 0
    cstart = np.zeros(NB, np.int64)
    coff = 0
    for I in range(NB):
        n = int(counts[I])
        gcol[dst_off:dst_off + n] = cols_s[src_off:src_off + n]
        dloc[dst_off:dst_off + n] = rows_s[src_off:src_off + n] - I * 128
        sval[dst_off:dst_off + n] = vals_s[src_off:src_off + n]
        cstart[I] = coff
        coff += int(nch[I])
        src_off += n
        dst_off += int(nch[I]) * 128
    NCH_TOT = coff
    assert dst_off == EP == NCH_TOT * 128

    # block id of each chunk
    chunk_blk = np.repeat(np.arange(NB), nch)

    # gather index tensor: position i -> (partition i%16 (replicated to 128),
    # free i//16)
    gidx = gcol.reshape(-1, 16).T.astype(np.int16)          # [16, EP/16]
    gidx = np.tile(gidx, (8, 1))                            # [128, EP/16]
    # dloc/sval: position i -> (partition i%128, chunk i//128)
    dloc_t = dloc.reshape(-1, 128).T.astype(ml_dtypes.bfloat16)  # [128, NCH]
    sval_t = sval.reshape(-1, 128).T.astype(ml_dtypes.bfloat16)  # [128, NCH]

    return dict(NB=NB, VP=VP, EP=EP, NCH=NCH_TOT, nch=nch, cstart=cstart,
                chunk_blk=chunk_blk, gidx=gidx, dloc=dloc_t, sval=sval_t)


def build_kernel(V, plan, R=4, BL=2, CIN=128, COUT=128):
    F = BL * CIN                   # 256
    NB, VP, EP, NCH = plan["NB"], plan["VP"], plan["EP"], plan["NCH"]
    nch, cstart, chunk_blk = plan["nch"], plan["cstart"], plan["chunk_blk"]
    NE_CHUNK = (V + 511) // 512    # einsum v-chunks (512 wide, last partial)

    nc = bacc.Bacc("TRN2", target_bir_lowering=False, debug=False,
                   num_swdge_queues=N_QUEUES)

    # ---- DRAM tensors ----
    xg0 = nc.dram_tensor("xg0", [VP, F], FP8, kind="ExternalInput")
    x0nm = nc.dram_tensor("x0nm", [VP, F], BF16, kind="ExternalInput")
    x0T = nc.dram_tensor("x0T", [F, VP], BF16, kind="ExternalInput")
    gidx = nc.dram_tensor("gidx", [128, EP // 16], I16, kind="ExternalInput")
    dlocd = nc.dram_tensor("dlocd", [128, NCH], BF16, kind="ExternalInput")
    svald = nc.dram_tensor("svald", [128, NCH], BF16, kind="ExternalInput")
    iotad = nc.dram_tensor("iotad", [128, SLICE_CH * 128], BF16,
                           kind="ExternalInput")
    identd = nc.dram_tensor("identd", [128, 128], BF16, kind="ExternalInput")
    wt = nc.dram_tensor("wt", [CIN, R, COUT], BF16, kind="ExternalInput")
    biasv = nc.dram_tensor("biasv", [COUT, 1], F32, kind="ExternalInput")
    yout = nc.dram_tensor("yout", [BL, COUT, V], F32, kind="ExternalOutput")

    xg = [nc.dram_tensor(f"xg{k}", [VP, F], FP8) for k in (1, 2)]
    xnm = [nc.dram_tensor(f"x{k}nm", [VP, F], BF16) for k in (1, 2)]
    xT = [nc.dram_tensor(f"x{k}T", [F, VP], BF16) for k in (1, 2, 3)]

    with tile.TileContext(nc, trace_sim=False) as tc:
        with (
            tc.tile_pool(name="res", bufs=1) as res,       # resident tables
            tc.tile_pool(name="ip", bufs=4) as ip,         # gather idx slices
            tc.tile_pool(name="gp", bufs=6) as gp,         # gathered msgs
            tc.tile_pool(name="sp", bufs=4) as sp,         # S blocks
            tc.tile_pool(name="pp", bufs=3, space="PSUM") as pp,   # y accum
            tc.tile_pool(name="tp", bufs=2, space="PSUM") as tp,   # transposes
            tc.tile_pool(name="cb", bufs=3) as cb,         # combine tiles
            tc.tile_pool(name="ep", bufs=8) as ep,         # einsum x tiles
            tc.tile_pool(name="pep", bufs=2, space="PSUM") as pep,  # einsum psum
            tc.tile_pool(name="eo", bufs=2) as eo,         # einsum out tiles
        ):
            # ---- resident tables ----
            dloc_sb = res.tile([128, NCH], BF16)
            nc.sync.dma_start(dloc_sb[:], dlocd[:])
            sval_sb = res.tile([128, NCH], BF16)
            nc.sync.dma_start(sval_sb[:], svald[:])
            iota_sb = res.tile([128, SLICE_CH * 128], BF16)
            nc.sync.dma_start(iota_sb[:], iotad[:])
            ident_sb = res.tile([128, 128], BF16)
            nc.sync.dma_start(ident_sb[:], identd[:])
            wts = res.tile([128, R, COUT], BF16)
            nc.sync.dma_start(wts[:], wt[:])
            bias_sb = res.tile([128, 1], F32)
            nc.sync.dma_start(bias_sb[:], biasv[:])

            gsrc = [xg0, xg[0], xg[1]]
            prevnm = [None, x0nm, xnm[0]]
            xnm_dst = [xnm[0], xnm[1], None]
            xg_dst = [xg[0], xg[1], None]
            xT_dst = [xT[0], xT[1], xT[2]]

            n_slice = (NCH + SLICE_CH - 1) // SLICE_CH

            for k in (0, 1, 2):       # chebyshev steps 1..3
                src_rows = gsrc[k][:]          # [VP, F] fp8
                # per-STORE_GRP group tiles (batched stores)
                grp_nm = None
                grp_f8 = None
                grp_tr = None
                for s in range(n_slice):
                    c0 = s * SLICE_CH
                    ncs = min(SLICE_CH, NCH - c0)
                    ne = ncs * 128
                    # gather idx slice
                    it = ip.tile([128, SLICE_CH * 8], I16, tag="idx")
                    nc.sync.dma_start(it[:, :ncs * 8],
                                      gidx[:, c0 * 8:c0 * 8 + ncs * 8])
                    # gather messages
                    mt = gp.tile([128, SLICE_CH, F], FP8, tag="msg")
                    nc.gpsimd.dma_gather(
                        mt[:, :ncs, :], src_rows, it[:, :ncs * 8],
                        num_idxs=ne, num_idxs_reg=ne, elem_size=F,
                        single_packet=False, queue_num=s % N_QUEUES)
                    # build S blocks for this slice
                    st = sp.tile([128, SLICE_CH, 128], BF16, tag="sblk")
                    nc.vector.tensor_tensor(
                        out=st[:, :ncs, :],
                        in0=dloc_sb[:, c0:c0 + ncs, None].to_broadcast(
                            [128, ncs, 128]),
                        in1=iota_sb[:].rearrange("p (c d) -> p c d", d=128)[
                            :, :ncs, :],
                        op=mybir.AluOpType.is_equal)
                    nc.vector.tensor_tensor(
                        out=st[:, :ncs, :],
                        in0=st[:, :ncs, :],
                        in1=sval_sb[:, c0:c0 + ncs, None].to_broadcast(
                            [128, ncs, 128]),
                        op=mybir.AluOpType.mult)
                    # scatter matmuls
                    for cl in range(ncs):
                        c = c0 + cl
                        I = int(chunk_blk[c])
                        first = (c == cstart[I])
                        last = (c == cstart[I] + nch[I] - 1)
                        if first:
                            ps = pp.tile([128, F], F32, tag="yac",
                                         name=f"yac_{k}_{I}")
                        nc.tensor.matmul(
                            ps[:], lhsT=st[:, cl, :], rhs=mt[:, cl, :],
                            start=first, stop=last)
                        if not last:
                            continue
                        # ---- combine block I ----
                        g = I % STORE_GRP
                        if g == 0:
                            ng = min(STORE_GRP, NB - I)
                            grp_nm = cb.tile([128, STORE_GRP, F], BF16,
                                             tag="gnm")
                            if k < 2:
                                grp_f8 = cb.tile([128, STORE_GRP, F], FP8,
                                                 tag="gf8")
                            grp_tr = cb.tile([128, STORE_GRP, 2, 128], BF16,
                                             tag="gtr")
                            if k > 0:
                                grp_pv = cb.tile([128, STORE_GRP, F], BF16,
                                                 tag="prev")
                                nc.sync.dma_start(
                                    grp_pv[:, :ng, :],
                                    prevnm[k][I * 128:I * 128 + ng * 128,
                                              :].rearrange(
                                        "(g p) f -> p g f", p=128))
                        if k == 0:
                            nc.scalar.activation(
                                grp_nm[:, g, :], ps[:],
                                mybir.ActivationFunctionType.Copy)
                        else:
                            t2 = cb.tile([128, F], BF16, tag="twoy")
                            nc.vector.tensor_scalar_mul(t2[:], ps[:], 2.0)
                            nc.vector.tensor_tensor(
                                out=grp_nm[:, g, :], in0=t2[:],
                                in1=grp_pv[:, g, :],
                                op=mybir.AluOpType.subtract)
                        if k < 2:
                            nc.scalar.activation(
                                grp_f8[:, g, :], grp_nm[:, g, :],
                                mybir.ActivationFunctionType.Copy)
                        for h in (0, 1):
                            tps = tp.tile([128, 128], BF16, tag="tps")
                            nc.tensor.transpose(
                                tps[:], grp_nm[:, g, h * 128:(h + 1) * 128],
                                ident_sb[:])
                            nc.scalar.activation(
                                grp_tr[:, g, h, :], tps[:],
                                mybir.ActivationFunctionType.Copy)
                        if g == ng - 1:
                            # batched stores for blocks I-g .. I
                            I0 = I - g
                            rsl = slice(I0 * 128, I0 * 128 + ng * 128)
                            if k < 2:
                                nc.scalar.dma_start(
                                    xnm_dst[k][rsl, :].rearrange(
                                        "(g p) f -> p g f", p=128),
                                    grp_nm[:, :ng, :])
                                nc.scalar.dma_start(
                                    xg_dst[k][rsl, :].rearrange(
                                        "(g p) f -> p g f", p=128),
                                    grp_f8[:, :ng, :])
                            for h in (0, 1):
                                nc.scalar.dma_start(
                                    xT_dst[k][h * 128:(h + 1) * 128,
                                              rsl].rearrange(
                                        "f (g d) -> f g d", d=128),
                                    grp_tr[:, :ng, h, :])

            # ---- einsum: out[b,o,v] = sum_r W_r^T @ x_r^T + bias ----
            xTs = [x0T, xT[0], xT[1], xT[2]]
            for b in range(BL):
                for chn in range(NE_CHUNK):
                    v0 = chn * 512
                    w = min(512, VP - v0)
                    wout = min(512, V - v0)
                    eps = pep.tile([128, 512], F32, tag="eps")
                    for r in range(R):
                        xt = ep.tile([128, 512], BF16, tag="ext")
                        nc.sync.dma_start(
                            xt[:, :w],
                            xTs[r][b * 128:(b + 1) * 128, v0:v0 + w])
                        nc.tensor.matmul(
                            eps[:, :w], lhsT=wts[:, r, :], rhs=xt[:, :w],
                            start=(r == 0), stop=(r == R - 1))
                    ob = eo.tile([128, 512], F32, tag="eob")
                    nc.vector.tensor_scalar_add(ob[:, :w], eps[:, :w],
                                                bias_sb[:])
                    nc.scalar.dma_start(yout[b, :, v0:v0 + wout],
                                        ob[:, :wout])

    nc.compile()
    return nc


def prep_inputs(x, weight, bias, lap_vals, lap_rows, lap_cols, plan):
    B, CIN, V = x.shape
    R = weight.shape[0]
    BL = B // N_CORES
    F = BL * CIN
    VP = plan["VP"]

    wt = np.ascontiguousarray(
        np.asarray(weight, np.float32).transpose(1, 0, 2)
    ).astype(ml_dtypes.bfloat16)                      # [CIN, R, COUT]
    biasv = np.asarray(bias, np.float32).reshape(-1, 1)
    iota = np.tile(np.arange(128, dtype=np.float32), SLICE_CH)
    iota = np.broadcast_to(iota, (128, SLICE_CH * 128)).astype(
        ml_dtypes.bfloat16)
    ident = np.eye(128, dtype=np.float32).astype(ml_dtypes.bfloat16)

    xf = np.asarray(x, np.float32)
    in_maps = []
    for c in range(N_CORES):
        xs = xf[c * BL:(c + 1) * BL]                    # (BL, CIN, V)
        x0 = np.transpose(xs, (2, 0, 1)).reshape(V, F)  # (V, F)
        x0p = np.zeros((VP, F), np.float32)
        x0p[:V] = x0
        x0b = x0p.astype(ml_dtypes.bfloat16)
        in_maps.append({
            "xg0": x0b.astype(NP_FP8),
            "x0nm": x0b,
            "x0T": np.ascontiguousarray(x0b.T),
            "gidx": plan["gidx"],
            "dlocd": plan["dloc"],
            "svald": plan["sval"],
            "iotad": np.ascontiguousarray(iota),
            "identd": ident,
            "wt": wt,
            "biasv": biasv,
        })
    return in_maps


_CACHE = {}


def get_built(x, weight, bias, lap_vals, lap_rows, lap_cols):
    V = x.shape[2]
    key = (V, len(lap_vals))
    if key not in _CACHE:
        plan = make_plan(lap_rows, lap_cols, lap_vals, V)
        nc = build_kernel(V, plan)
        _CACHE[key] = (nc, plan)
    return _CACHE[key]


def kernel(x, weight, bias, lap_vals, lap_rows, lap_cols):
    B, CIN, V = x.shape
    nc, plan = get_built(x, weight, bias, lap_vals, lap_rows, lap_cols)
    in_maps = prep_inputs(x, weight, bias, lap_vals, lap_rows, lap_cols, plan)
    res = bass_utils.run_bass_kernel_spmd(
        nc, in_maps, core_ids=list(range(N_CORES)))
    out = np.concatenate([res.results[c]["yout"] for c in range(N_CORES)],
                         axis=0)
    return out.astype(np.float32)


if __name__ == "__main__":
    V, NNZ, B, CIN, COUT, R = 1024, 32768, 16, 128, 128, 4
    rng = np.random.default_rng(0)
    x = rng.standard_normal((B, CIN, V)).astype(np.float32)
    weight = (rng.standard_normal((R, CIN, COUT)) *
              np.sqrt(2.0 / (R * CIN))).astype(np.float32)
    bias = np.full((COUT,), 0.01, np.float32)
    lap_vals = (rng.standard_normal(NNZ) / 32.0).astype(np.float32)
    lap_rows = rng.integers(0, V, NNZ).astype(np.int32)
    lap_cols = rng.integers(0, V, NNZ).astype(np.int32)

    def ref(x, weight, bias, lv, lr, lc):
        Vd_ = x.shape[2]
        L = np.zeros((Vd_, Vd_), np.float64)
        np.add.at(L, (lr, lc), lv.astype(np.float64))
        x0 = np.transpose(x, (2, 0, 1)).reshape(Vd_, -1).astype(np.float64)
        xs = [x0, L @ x0]
        for _ in range(R - 2):
            xs.append(2.0 * (L @ xs[-1]) - xs[-2])
        xs = np.stack(xs).reshape(R, Vd_, B, CIN)
        out = np.einsum('rvbi,rio->vbo', xs, weight.astype(np.float64))
        out = out + bias
        return np.transpose(out, (1, 2, 0)).astype(np.float32)

    expected = ref(x, weight, bias, lap_vals, lap_rows, lap_cols)
    got = kernel(x, weight, bias, lap_vals, lap_rows, lap_cols)
    err = np.abs(got - expected)
    scale = np.abs(expected).max()
    print("max abs err:", err.max(), "scale:", scale,
          "rel:", err.max() / scale)
